# revision 30
# baseline (speedup 1.0000x reference)
"""Multi-head attention layer (B=2, L=S=4096, E=512, H=8, hd=64) on 8 TRN2
NeuronCores.  ~380us HW exec (baseline 613us), rel err 4.1e-3.

Sharding (no collectives): core c handles batch b=c//4 and query rows
[(c%4)*1024, (c%4+1)*1024). Each core projects the full K/V of its batch
(duplicated across the 4 cores of a batch group — an AllGather dedup was
tried and lost: ~60-70us collective latency gates attention start), plus
its own Q slice, runs flash-style attention, and the output projection for
its rows. Host assembles the 8 slices.

ACT(exp) is the pacing engine: 33.5M score elements/core at ~1.09-1.2
ns/column, ~280us busy, and exp is ACT-only (DVE/GpSimd have no exp;
Schraudolph-on-DVE and fp8 PV/V-proj all fail the 2e-2 gate because this
problem's attention is highly peaked, so per-element p/v errors do not
average out).  Everything else hides under ACT:
- q/k/v transposed + cast to bf16 on the HOST: no PE input transposes, no
  DVE casts, half the input DMA bytes; output written feature-major from
  out-proj PSUM, host transposes back and adds the folded bias (bo+Wo@bv).
- per 128-key chunk: 2 score matmuls (the 2 heads of a pair run
  concurrently on different PE row-quadrants via tile_position), one
  N=1024 exp on ACT, 2 accumulating PV matmuls (stationary [vh|ones]
  gives the softmax denominator for free in PSUM partition 64).
- all projections are software-pipelined into the attention stream as
  fillers (V proj leads its consuming chunk by 3; K proj for head-pair
  hp+1 runs during hp's loop; Q proj for query-group 1 and the out-proj
  of group 0 run inside later groups), so ACT goes dense from ~30us.
- PSUM: sab double-buffer 4 banks, single pv accumulator 2 banks
  (evacuated to SBUF at group end so the WAR stall is one DVE copy),
  proj ping-pong 2 banks.
- normalization SBUF-side: reciprocal_approx_fast (~18-bit) + GpSimd
  partition_broadcast + DVE multiply.  NOTE: custom DVE ops silently drop
  the partition offset of their input AP — rowsums are first copied to
  partition 0.
- softmax without max-subtraction (scaled scores bounded ~1.7 here).
"""

import numpy as np
import ml_dtypes

import concourse.bass as bass
import concourse.mybir as mybir
import concourse.tile as tile
from concourse import bacc
from concourse.bass_utils import run_bass_kernel_spmd

F32 = mybir.dt.float32
BF16 = mybir.dt.bfloat16
EXP = mybir.ActivationFunctionType.Exp
ADD = mybir.AluOpType.add
MULT = mybir.AluOpType.mult

B, L, E, H = 2, 4096, 512, 8
HD = E // H            # 64
N_CORES = 8
LLOC = B * L // N_CORES  # 1024 query rows per core
SCALE = HD ** -0.5       # 0.125

NQG = LLOC // 512   # 2 query groups of 512 rows
NSG = L // 512      # 8 key/value groups of 512 rows
NSC = L // 128      # 32 key chunks of 128

_STATE = {}


def ts(i, n):
    return bass.ts(i, n)


def _build():
    nc = bacc.Bacc("TRN2", target_bir_lowering=False, debug=False,
                   num_devices=N_CORES)

    q_d = nc.dram_tensor("qt", [E, LLOC], BF16, kind="ExternalInput")
    k_d = nc.dram_tensor("kt", [E, L], BF16, kind="ExternalInput")
    v_d = nc.dram_tensor("vt", [E, L], BF16, kind="ExternalInput")
    wq_d = nc.dram_tensor("wqt", [E, E], BF16, kind="ExternalInput")
    wk_d = nc.dram_tensor("wkt", [E, E], BF16, kind="ExternalInput")
    wv_d = nc.dram_tensor("wvt", [E, E], BF16, kind="ExternalInput")
    wo_d = nc.dram_tensor("wot", [E, E], BF16, kind="ExternalInput")
    bq_d = nc.dram_tensor("bq", [E], F32, kind="ExternalInput")
    bk_d = nc.dram_tensor("bk", [E], F32, kind="ExternalInput")
    out_d = nc.dram_tensor("out", [E, LLOC], F32, kind="ExternalOutput")

    with tile.TileContext(nc) as tc:
        with (
            tc.tile_pool(name="consts", bufs=1) as consts,
            tc.tile_pool(name="big", bufs=1) as big,
            tc.tile_pool(name="qstg", bufs=1) as qstg_p,
            tc.tile_pool(name="kvstg", bufs=2) as kvstg_p,
            tc.tile_pool(name="pab", bufs=2) as pab_p,
            tc.tile_pool(name="pvs", bufs=2) as pvs_p,
            tc.tile_pool(name="rv", bufs=1) as rv_p,
            tc.tile_pool(name="rrep", bufs=1) as rrep_p,
            tc.tile_pool(name="yt", bufs=1) as yt_p,
            tc.tile_pool(name="ps_proj", bufs=2, space="PSUM") as ps_proj,
            tc.tile_pool(name="ps_sab", bufs=2, space="PSUM") as ps_sab,
            tc.tile_pool(name="ps_pv", bufs=1, space="PSUM") as ps_pv,
        ):
            # ---------------- weights / biases (DMA only) ----------------
            # w*_sb[p, ci, o] = W[o, ci*128+p] = WT[ci*128+p, o]
            wq_sb = consts.tile([128, 4, E], BF16, tag="wq")
            for ci in range(4):
                nc.sync.dma_start(wq_sb[:, ci, :], wq_d.ap()[ts(ci, 128), :])
            bqt = consts.tile([128, 4], F32, tag="bqt")
            nc.sync.dma_start(bqt[:], bq_d.ap().rearrange("(c p) -> p c", p=128))

            # ---------------- big resident tensors ----------------
            qht = big.tile([128, 4, LLOC], BF16, tag="qht")
            kht = big.tile([128, 4, L], BF16, tag="kht")
            vha = big.tile([128, NSC, H * (HD + 1)], BF16, tag="vha")
            nc.vector.memset(
                vha[:].rearrange("p c (h x) -> p c h x", x=HD + 1)[:, :, :, HD:HD + 1],
                1.0)
            att = big.tile([64, H, LLOC], BF16, tag="att")

            # ---------------- staging DMAs ----------------
            qstg = qstg_p.tile([128, 4, LLOC], BF16, tag="qstg")
            for ci in range(4):
                for h2 in range(2):
                    nc.sync.dma_start(
                        qstg[:, ci, ts(h2, 512)],
                        q_d.ap()[ts(ci, 128), ts(h2, 512)])
            bkt = consts.tile([128, 4], F32, tag="bkt")
            nc.sync.dma_start(bkt[:], bk_d.ap().rearrange("(c p) -> p c", p=128))
            wk_sb = consts.tile([128, 4, E], BF16, tag="wk")
            wv_sb = consts.tile([128, 4, E], BF16, tag="wv")
            for w_sb, w_d in ((wk_sb, wk_d), (wv_sb, wv_d)):
                for ci in range(4):
                    nc.sync.dma_start(w_sb[:, ci, :], w_d.ap()[ts(ci, 128), :])
            kstg = kvstg_p.tile([128, 4, L], BF16, tag="kv")
            vstg = kvstg_p.tile([128, 4, L], BF16, tag="kv")
            for cc in range(4):
                for ci in range(4):
                    nc.sync.dma_start(
                        kstg[:, ci, ts(cc, 1024)],
                        k_d.ap()[ts(ci, 128), ts(cc, 1024)])
                for ci in range(4):
                    nc.sync.dma_start(
                        vstg[:, ci, ts(cc, 1024)],
                        v_d.ap()[ts(ci, 128), ts(cc, 1024)])

            # wo needed only for out-proj: issue after the big input DMAs
            # wo_sb[d, h, o] = Wo[o, h*64+d] = WoT[h*64+d, o]
            wo_sb = consts.tile([64, H, E], BF16, tag="wo")
            for h in range(H):
                nc.sync.dma_start(wo_sb[:, h, :], wo_d.ap()[ts(h, 64), :])

            # ---------------- projection emitters ----------------
            def q_group(mg, co):
                pp = ps_proj.tile([128, 512], F32, tag="pp")
                for ci in range(4):
                    nc.tensor.matmul(pp[:], wq_sb[:, ci, ts(co, 128)],
                                     qstg[:, ci, ts(mg, 512)],
                                     start=(ci == 0), stop=(ci == 3))
                nc.vector.tensor_scalar(
                    out=qht[:, co, ts(mg, 512)], in0=pp[:],
                    scalar1=bqt[:, co:co + 1], scalar2=None, op0=ADD)

            def k_group(hp, g):
                pp = ps_proj.tile([128, 512], F32, tag="pp")
                for ci in range(4):
                    nc.tensor.matmul(pp[:], wk_sb[:, ci, ts(hp, 128)],
                                     kstg[:, ci, ts(g, 512)],
                                     start=(ci == 0), stop=(ci == 3))
                nc.vector.tensor_scalar(
                    out=kht[:, hp, ts(g, 512)], in0=pp[:],
                    scalar1=bkt[:, hp:hp + 1], scalar2=None, op0=ADD)

            def v_group(sc):
                pp = ps_proj.tile([128, 512], F32, tag="pp")
                for ci in range(4):
                    nc.tensor.matmul(pp[:], vstg[:, ci, ts(sc, 128)],
                                     wv_sb[:, ci, :],
                                     start=(ci == 0), stop=(ci == 3))
                nc.vector.tensor_copy(
                    vha[:, sc, :].rearrange("p (h x) -> p h x", x=HD + 1)[:, :, 0:HD],
                    pp[:].rearrange("p (h d) -> p h d", d=HD))

            def outproj_y(mg, co, tail=False):
                Y = ps_proj.tile([128, 512], F32, tag="pp")
                for h in range(H):
                    nc.tensor.matmul(Y[:], wo_sb[:, h, ts(co, 128)],
                                     att[:, h, ts(mg, 512)],
                                     start=(h == 0), stop=(h == H - 1))
                yt = yt_p.tile([128, 512], F32, tag="yt")
                if tail:  # ACT is idle in the tail; keep DVE off the chain
                    nc.scalar.copy(yt[:], Y[:])
                else:
                    nc.vector.tensor_copy(yt[:], Y[:])
                nc.sync.dma_start(out_d.ap()[ts(co, 128), ts(mg, 512)], yt[:])

            # ---------------- upfront projections ----------------
            for co in range(2):
                q_group(0, co)
            for g in range(2):
                k_group(0, g)
            for co in range(2, 4):
                q_group(0, co)
            for sc in range(3):
                v_group(sc)

            # ---------------- attention (mg outer, head-pair inner) ---------
            def scores_exp(mg, hp, sc):
                sab = ps_sab.tile([128, 2, 512], F32, tag="sab")
                nc.tensor.matmul(sab[:, 0, :],
                                 kht[0:64, hp, ts(sc, 128)],
                                 qht[0:64, hp, ts(mg, 512)],
                                 start=True, stop=True,
                                 tile_position=(0, 0))
                nc.tensor.matmul(sab[:, 1, :],
                                 kht[64:128, hp, ts(sc, 128)],
                                 qht[64:128, hp, ts(mg, 512)],
                                 start=True, stop=True,
                                 tile_position=(64, 0))
                pab = pab_p.tile([128, 2, 512], BF16, tag="pab")
                nc.scalar.activation(pab[:], sab[:], EXP, scale=SCALE)
                return pab

            def pv_pair(hp, sc, pv, pab):
                hA, hB = 2 * hp, 2 * hp + 1
                nc.tensor.matmul(pv[:, 0, :],
                                 vha[:, sc, hA * 65: hA * 65 + 65],
                                 pab[:, 0, :],
                                 start=(sc == 0), stop=(sc == NSC - 1))
                nc.tensor.matmul(pv[:, 1, :],
                                 vha[:, sc, hB * 65: hB * 65 + 65],
                                 pab[:, 1, :],
                                 start=(sc == 0), stop=(sc == NSC - 1))

            groups = [(mg, hp) for mg in range(NQG) for hp in range(4)]
            pab_carry = None  # next group's chunk-0 pab, emitted at boundary
            for gi, (mg, hp) in enumerate(groups):
                hA, hB = 2 * hp, 2 * hp + 1
                pv = ps_pv.tile([65, 2, 512], F32, tag="pv")
                for sc in range(NSC):
                    # interleaved projection / out-proj fillers
                    if mg == 0:
                        if hp == 0:
                            if sc + 3 < NSC:
                                v_group(sc + 3)
                            if sc % 4 == 0 and sc // 4 < 6:
                                k_group(0, 2 + sc // 4)
                            if sc % 4 == 2:
                                k_group(1, sc // 4)
                        elif hp == 1 and sc % 4 == 0:
                            k_group(2, sc // 4)
                        elif hp == 2 and sc % 4 == 0:
                            k_group(3, sc // 4)
                        elif hp == 3 and sc % 8 == 0:
                            q_group(1, sc // 8)
                    elif hp == 0 and sc % 8 == 4:
                        outproj_y(0, sc // 8)
                    # attention chunk (chunk 0's scores may be carried in
                    # from the previous group's boundary)
                    if sc == 0 and pab_carry is not None:
                        pab = pab_carry
                        pab_carry = None
                    else:
                        pab = scores_exp(mg, hp, sc)
                    if sc == NSC - 1 and gi + 1 < len(groups):
                        nmg, nhp = groups[gi + 1]
                        pab_carry = scores_exp(nmg, nhp, 0)
                    pv_pair(hp, sc, pv, pab)
                    # evacuate pv to SBUF (frees the PSUM accumulator fast),
                    # then normalize SBUF-side:
                    # att[:, h, mg] = pvs[0:64] * (1/rowsum)
                    pvs = pvs_p.tile([65, 2, 512], F32, tag="pvs")
                    nc.vector.tensor_copy(pvs[:], pv[:])
                    # rowsum to partition 0 (custom DVE ops drop the
                    # partition offset of their input AP)
                    rs = rv_p.tile([1, 2, 512], F32, tag="rs")
                    nc.vector.tensor_copy(rs[:], pvs[64:65, :, :])
                    rv = rv_p.tile([1, 2, 512], F32, tag="rv")
                    nc.vector.reciprocal_approx_fast(out=rv[:], in_=rs[:])
                    rrep = rrep_p.tile([64, 2, 512], F32, tag="rrep")
                    nc.gpsimd.partition_broadcast(rrep[:], rv[:])
                    for i, h in ((0, hA), (1, hB)):
                        nc.vector.tensor_tensor(
                            out=att[:, h, ts(mg, 512)], in0=pvs[0:64, i, :],
                            in1=rrep[:, i, :], op=MULT)

            # ---------------- tail: out-proj for mg1 ----------------
            for co in range(4):
                outproj_y(1, co, tail=True)

    nc.compile()
    return nc


def _get_nc():
    if "nc" not in _STATE:
        _STATE["nc"] = _build()
    return _STATE["nc"]


def _bf16(x):
    return np.ascontiguousarray(x.astype(ml_dtypes.bfloat16))


def _shard(inputs):
    q = np.asarray(inputs["q"], dtype=np.float32)
    k = np.asarray(inputs["k"], dtype=np.float32)
    v = np.asarray(inputs["v"], dtype=np.float32)
    WqT = _bf16(np.asarray(inputs["Wq"], np.float32).T)
    WkT = _bf16(np.asarray(inputs["Wk"], np.float32).T)
    WvT = _bf16(np.asarray(inputs["Wv"], np.float32).T)
    WoT = _bf16(np.asarray(inputs["Wo"], np.float32).T)
    bq = np.asarray(inputs["bq"], np.float32)
    bk = np.asarray(inputs["bk"], np.float32)

    kT = [_bf16(k[b].T) for b in range(B)]
    vT = [_bf16(v[b].T) for b in range(B)]

    in_maps = []
    for c in range(N_CORES):
        b, j = divmod(c, N_CORES // B)
        in_maps.append({
            "qt": _bf16(q[b, j * LLOC:(j + 1) * LLOC].T),
            "kt": kT[b],
            "vt": vT[b],
            "wqt": WqT, "wkt": WkT, "wvt": WvT, "wot": WoT,
            "bq": bq, "bk": bk,
        })
    return in_maps


def _run(inputs, trace=False):
    nc = _get_nc()
    in_maps = _shard(inputs)
    res = run_bass_kernel_spmd(nc, in_maps, core_ids=list(range(N_CORES)),
                               trace=trace)
    # v-bias commutes through attention (rows of P sum to 1 after
    # normalization): fold Wo @ bv into the output bias, added on host.
    Wo = np.asarray(inputs["Wo"], np.float32)
    bo_eff = (np.asarray(inputs["bo"], np.float32)
              + Wo @ np.asarray(inputs["bv"], np.float32))
    out = np.empty((B, L, E), np.float32)
    for c in range(N_CORES):
        b, j = divmod(c, N_CORES // B)
        out[b, j * LLOC:(j + 1) * LLOC] = res.results[c]["out"].T + bo_eff
    return out, res


def kernel(**inputs) -> np.ndarray:
    return _run(inputs)[0]


# revision 31
# speedup vs baseline: 3.9289x; 3.9289x over previous
"""Multi-head attention layer (B=2, L=S=4096, E=512, H=8, hd=64) on 8 TRN2
NeuronCores.  ~380us HW exec (baseline 613us), rel err 4.1e-3.

Sharding (no collectives): core c handles batch b=c//4 and query rows
[(c%4)*1024, (c%4+1)*1024). Each core projects the full K/V of its batch
(duplicated across the 4 cores of a batch group — an AllGather dedup was
tried and lost: ~60-70us collective latency gates attention start), plus
its own Q slice, runs flash-style attention, and the output projection for
its rows. Host assembles the 8 slices.

ACT(exp) is the pacing engine: 33.5M score elements/core at ~1.09-1.2
ns/column, ~280us busy, and exp is ACT-only (DVE/GpSimd have no exp;
Schraudolph-on-DVE and fp8 PV/V-proj all fail the 2e-2 gate because this
problem's attention is highly peaked, so per-element p/v errors do not
average out).  Everything else hides under ACT:
- q/k/v transposed + cast to bf16 on the HOST: no PE input transposes, no
  DVE casts, half the input DMA bytes; output written feature-major from
  out-proj PSUM, host transposes back and adds the folded bias (bo+Wo@bv).
- per 128-key chunk: 2 score matmuls (the 2 heads of a pair run
  concurrently on different PE row-quadrants via tile_position), one
  N=1024 exp on ACT, 2 accumulating PV matmuls (stationary [vh|ones]
  gives the softmax denominator for free in PSUM partition 64).
- all projections are software-pipelined into the attention stream as
  fillers (V proj leads its consuming chunk by 3; K proj for head-pair
  hp+1 runs during hp's loop; Q proj for query-group 1 and the out-proj
  of group 0 run inside later groups), so ACT goes dense from ~30us.
- PSUM: sab double-buffer 4 banks, single pv accumulator 2 banks
  (evacuated to SBUF at group end so the WAR stall is one DVE copy),
  proj ping-pong 2 banks.
- normalization SBUF-side: reciprocal_approx_fast (~18-bit) + GpSimd
  partition_broadcast + DVE multiply.  NOTE: custom DVE ops silently drop
  the partition offset of their input AP — rowsums are first copied to
  partition 0.
- softmax without max-subtraction (scaled scores bounded ~1.7 here).
"""

import numpy as np
import ml_dtypes

import concourse.bass as bass
import concourse.mybir as mybir
import concourse.tile as tile
from concourse import bacc
from concourse.bass_utils import run_bass_kernel_spmd

F32 = mybir.dt.float32
BF16 = mybir.dt.bfloat16
EXP = mybir.ActivationFunctionType.Exp
ADD = mybir.AluOpType.add
MULT = mybir.AluOpType.mult

B, L, E, H = 2, 4096, 512, 8
HD = E // H            # 64
N_CORES = 8
LLOC = B * L // N_CORES  # 1024 query rows per core
SCALE = HD ** -0.5       # 0.125

NQG = LLOC // 512   # 2 query groups of 512 rows
NSG = L // 512      # 8 key/value groups of 512 rows
NSC = L // 128      # 32 key chunks of 128

_STATE = {}


def ts(i, n):
    return bass.ts(i, n)


def _build():
    nc = bacc.Bacc("TRN2", target_bir_lowering=False, debug=False,
                   num_devices=N_CORES)

    q_d = nc.dram_tensor("qt", [E, LLOC], BF16, kind="ExternalInput")
    k_d = nc.dram_tensor("kt", [E, L], BF16, kind="ExternalInput")
    v_d = nc.dram_tensor("vt", [E, L], BF16, kind="ExternalInput")
    wq_d = nc.dram_tensor("wqt", [E, E], BF16, kind="ExternalInput")
    wk_d = nc.dram_tensor("wkt", [E, E], BF16, kind="ExternalInput")
    wv_d = nc.dram_tensor("wvt", [E, E], BF16, kind="ExternalInput")
    wo_d = nc.dram_tensor("wot", [E, E], BF16, kind="ExternalInput")
    bq_d = nc.dram_tensor("bq", [E], F32, kind="ExternalInput")
    bk_d = nc.dram_tensor("bk", [E], F32, kind="ExternalInput")
    out_d = nc.dram_tensor("out", [E, LLOC], F32, kind="ExternalOutput")

    with tile.TileContext(nc) as tc:
        with (
            tc.tile_pool(name="consts", bufs=1) as consts,
            tc.tile_pool(name="big", bufs=1) as big,
            tc.tile_pool(name="qstg", bufs=1) as qstg_p,
            tc.tile_pool(name="kvstg", bufs=2) as kvstg_p,
            tc.tile_pool(name="pab", bufs=2) as pab_p,
            tc.tile_pool(name="pvs", bufs=2) as pvs_p,
            tc.tile_pool(name="rv", bufs=1) as rv_p,
            tc.tile_pool(name="rrep", bufs=1) as rrep_p,
            tc.tile_pool(name="yt", bufs=1) as yt_p,
            tc.tile_pool(name="ps_proj", bufs=2, space="PSUM") as ps_proj,
            tc.tile_pool(name="ps_sab", bufs=2, space="PSUM") as ps_sab,
            tc.tile_pool(name="ps_pv", bufs=1, space="PSUM") as ps_pv,
        ):
            # ---------------- weights / biases (DMA only) ----------------
            # w*_sb[p, ci, o] = W[o, ci*128+p] = WT[ci*128+p, o]
            wq_sb = consts.tile([128, 4, E], BF16, tag="wq")
            for ci in range(4):
                nc.sync.dma_start(wq_sb[:, ci, :], wq_d.ap()[ts(ci, 128), :])
            bqt = consts.tile([128, 4], F32, tag="bqt")
            nc.sync.dma_start(bqt[:], bq_d.ap().rearrange("(c p) -> p c", p=128))

            # ---------------- big resident tensors ----------------
            qht = big.tile([128, 4, LLOC], BF16, tag="qht")
            kht = big.tile([128, 4, L], BF16, tag="kht")
            vha = big.tile([128, NSC, H * (HD + 1)], BF16, tag="vha")
            nc.vector.memset(
                vha[:].rearrange("p c (h x) -> p c h x", x=HD + 1)[:, :, :, HD:HD + 1],
                1.0)
            att = big.tile([64, H, LLOC], BF16, tag="att")

            # ---------------- staging DMAs ----------------
            qstg = qstg_p.tile([128, 4, LLOC], BF16, tag="qstg")
            for ci in range(4):
                for h2 in range(2):
                    nc.sync.dma_start(
                        qstg[:, ci, ts(h2, 512)],
                        q_d.ap()[ts(ci, 128), ts(h2, 512)])
            bkt = consts.tile([128, 4], F32, tag="bkt")
            nc.sync.dma_start(bkt[:], bk_d.ap().rearrange("(c p) -> p c", p=128))
            wk_sb = consts.tile([128, 4, E], BF16, tag="wk")
            wv_sb = consts.tile([128, 4, E], BF16, tag="wv")
            for w_sb, w_d in ((wk_sb, wk_d), (wv_sb, wv_d)):
                for ci in range(4):
                    nc.sync.dma_start(w_sb[:, ci, :], w_d.ap()[ts(ci, 128), :])
            kstg = kvstg_p.tile([128, 4, L], BF16, tag="kv")
            vstg = kvstg_p.tile([128, 4, L], BF16, tag="kv")
            for cc in range(4):
                for ci in range(4):
                    nc.sync.dma_start(
                        kstg[:, ci, ts(cc, 1024)],
                        k_d.ap()[ts(ci, 128), ts(cc, 1024)])
                for ci in range(4):
                    nc.sync.dma_start(
                        vstg[:, ci, ts(cc, 1024)],
                        v_d.ap()[ts(ci, 128), ts(cc, 1024)])

            # wo needed only for out-proj: issue after the big input DMAs
            # wo_sb[d, h, o] = Wo[o, h*64+d] = WoT[h*64+d, o]
            wo_sb = consts.tile([64, H, E], BF16, tag="wo")
            for h in range(H):
                nc.sync.dma_start(wo_sb[:, h, :], wo_d.ap()[ts(h, 64), :])

            # ---------------- projection emitters ----------------
            def q_group(mg, co):
                pp = ps_proj.tile([128, 512], F32, tag="pp")
                for ci in range(4):
                    nc.tensor.matmul(pp[:], wq_sb[:, ci, ts(co, 128)],
                                     qstg[:, ci, ts(mg, 512)],
                                     start=(ci == 0), stop=(ci == 3))
                nc.vector.tensor_scalar(
                    out=qht[:, co, ts(mg, 512)], in0=pp[:],
                    scalar1=bqt[:, co:co + 1], scalar2=None, op0=ADD)

            def k_group(hp, g):
                pp = ps_proj.tile([128, 512], F32, tag="pp")
                for ci in range(4):
                    nc.tensor.matmul(pp[:], wk_sb[:, ci, ts(hp, 128)],
                                     kstg[:, ci, ts(g, 512)],
                                     start=(ci == 0), stop=(ci == 3))
                nc.vector.tensor_scalar(
                    out=kht[:, hp, ts(g, 512)], in0=pp[:],
                    scalar1=bkt[:, hp:hp + 1], scalar2=None, op0=ADD)

            def v_group(sc):
                pp = ps_proj.tile([128, 512], F32, tag="pp")
                for ci in range(4):
                    nc.tensor.matmul(pp[:], vstg[:, ci, ts(sc, 128)],
                                     wv_sb[:, ci, :],
                                     start=(ci == 0), stop=(ci == 3))
                nc.vector.tensor_copy(
                    vha[:, sc, :].rearrange("p (h x) -> p h x", x=HD + 1)[:, :, 0:HD],
                    pp[:].rearrange("p (h d) -> p h d", d=HD))

            def outproj_y(mg, co, tail=False):
                Y = ps_proj.tile([128, 512], F32, tag="pp")
                for h in range(H):
                    nc.tensor.matmul(Y[:], wo_sb[:, h, ts(co, 128)],
                                     att[:, h, ts(mg, 512)],
                                     start=(h == 0), stop=(h == H - 1))
                yt = yt_p.tile([128, 512], F32, tag="yt")
                if tail:  # ACT is idle in the tail; keep DVE off the chain
                    nc.scalar.copy(yt[:], Y[:])
                else:
                    nc.vector.tensor_copy(yt[:], Y[:])
                nc.sync.dma_start(out_d.ap()[ts(co, 128), ts(mg, 512)], yt[:])

            # ---------------- upfront projections ----------------
            for co in range(2):
                q_group(0, co)
            for g in range(2):
                k_group(0, g)
            for co in range(2, 4):
                q_group(0, co)
            for sc in range(3):
                v_group(sc)

            # ---------------- attention (mg outer, head-pair inner) ---------
            for mg in range(NQG):
                for hp in range(4):
                    hA, hB = 2 * hp, 2 * hp + 1
                    pv = ps_pv.tile([65, 2, 512], F32, tag="pv")
                    for sc in range(NSC):
                        # interleaved projection / out-proj fillers
                        if mg == 0:
                            if hp == 0:
                                if sc + 3 < NSC:
                                    v_group(sc + 3)
                                if sc % 4 == 0 and sc // 4 < 6:
                                    k_group(0, 2 + sc // 4)
                                if sc % 4 == 2:
                                    k_group(1, sc // 4)
                            elif hp == 1 and sc % 4 == 0:
                                k_group(2, sc // 4)
                            elif hp == 2 and sc % 4 == 0:
                                k_group(3, sc // 4)
                            elif hp == 3 and sc % 8 == 0:
                                q_group(1, sc // 8)
                        elif hp == 0 and sc % 8 == 4:
                            outproj_y(0, sc // 8)
                        # attention chunk
                        sab = ps_sab.tile([128, 2, 512], F32, tag="sab")
                        nc.tensor.matmul(sab[:, 0, :],
                                         kht[0:64, hp, ts(sc, 128)],
                                         qht[0:64, hp, ts(mg, 512)],
                                         start=True, stop=True,
                                         tile_position=(0, 0))
                        nc.tensor.matmul(sab[:, 1, :],
                                         kht[64:128, hp, ts(sc, 128)],
                                         qht[64:128, hp, ts(mg, 512)],
                                         start=True, stop=True,
                                         tile_position=(64, 0))
                        pab = pab_p.tile([128, 2, 512], BF16, tag="pab")
                        nc.scalar.activation(pab[:], sab[:], EXP, scale=SCALE)
                        nc.tensor.matmul(pv[:, 0, :],
                                         vha[:, sc, hA * 65: hA * 65 + 65],
                                         pab[:, 0, :],
                                         start=(sc == 0), stop=(sc == NSC - 1))
                        nc.tensor.matmul(pv[:, 1, :],
                                         vha[:, sc, hB * 65: hB * 65 + 65],
                                         pab[:, 1, :],
                                         start=(sc == 0), stop=(sc == NSC - 1))
                    # evacuate pv to SBUF (frees the PSUM accumulator fast),
                    # then normalize SBUF-side:
                    # att[:, h, mg] = pvs[0:64] * (1/rowsum)
                    pvs = pvs_p.tile([65, 2, 512], F32, tag="pvs")
                    nc.vector.tensor_copy(pvs[:], pv[:])
                    # rowsum to partition 0 (custom DVE ops drop the
                    # partition offset of their input AP)
                    rs = rv_p.tile([1, 2, 512], F32, tag="rs")
                    nc.vector.tensor_copy(rs[:], pvs[64:65, :, :])
                    rv = rv_p.tile([1, 2, 512], F32, tag="rv")
                    nc.vector.reciprocal_approx_fast(out=rv[:], in_=rs[:])
                    rrep = rrep_p.tile([64, 2, 512], F32, tag="rrep")
                    nc.gpsimd.partition_broadcast(rrep[:], rv[:])
                    for i, h in ((0, hA), (1, hB)):
                        nc.vector.tensor_tensor(
                            out=att[:, h, ts(mg, 512)], in0=pvs[0:64, i, :],
                            in1=rrep[:, i, :], op=MULT)

            # ---------------- tail: out-proj for mg1 ----------------
            for co in range(4):
                outproj_y(1, co, tail=True)

    nc.compile()
    return nc


def _get_nc():
    if "nc" not in _STATE:
        _STATE["nc"] = _build()
    return _STATE["nc"]


def _bf16(x):
    return np.ascontiguousarray(x.astype(ml_dtypes.bfloat16))


def _shard(inputs):
    q = np.asarray(inputs["q"], dtype=np.float32)
    k = np.asarray(inputs["k"], dtype=np.float32)
    v = np.asarray(inputs["v"], dtype=np.float32)
    WqT = _bf16(np.asarray(inputs["Wq"], np.float32).T)
    WkT = _bf16(np.asarray(inputs["Wk"], np.float32).T)
    WvT = _bf16(np.asarray(inputs["Wv"], np.float32).T)
    WoT = _bf16(np.asarray(inputs["Wo"], np.float32).T)
    bq = np.asarray(inputs["bq"], np.float32)
    bk = np.asarray(inputs["bk"], np.float32)

    kT = [_bf16(k[b].T) for b in range(B)]
    vT = [_bf16(v[b].T) for b in range(B)]

    in_maps = []
    for c in range(N_CORES):
        b, j = divmod(c, N_CORES // B)
        in_maps.append({
            "qt": _bf16(q[b, j * LLOC:(j + 1) * LLOC].T),
            "kt": kT[b],
            "vt": vT[b],
            "wqt": WqT, "wkt": WkT, "wvt": WvT, "wot": WoT,
            "bq": bq, "bk": bk,
        })
    return in_maps


def _run(inputs, trace=False):
    nc = _get_nc()
    in_maps = _shard(inputs)
    res = run_bass_kernel_spmd(nc, in_maps, core_ids=list(range(N_CORES)),
                               trace=trace)
    # v-bias commutes through attention (rows of P sum to 1 after
    # normalization): fold Wo @ bv into the output bias, added on host.
    Wo = np.asarray(inputs["Wo"], np.float32)
    bo_eff = (np.asarray(inputs["bo"], np.float32)
              + Wo @ np.asarray(inputs["bv"], np.float32))
    out = np.empty((B, L, E), np.float32)
    for c in range(N_CORES):
        b, j = divmod(c, N_CORES // B)
        out[b, j * LLOC:(j + 1) * LLOC] = res.results[c]["out"].T + bo_eff
    return out, res


def kernel(**inputs) -> np.ndarray:
    return _run(inputs)[0]
